# revision 10
# baseline (speedup 1.0000x reference)
"""MoE grouped-GEMM expert MLP for Trainium2, expert-parallel over 8 NeuronCores.

Problem: x:(B=2, E=8, N=2048, D=1024), per-expert 2-layer GELU MLP with
w1:(E, D, F=4096), w2:(E, F, D).  Reference computes
  xe = x.reshape(E, B*N, D)          # pure buffer reinterpretation
  h  = gelu_tanh(xe @ w1 + b1)
  out= h @ w2 + b2                   # reshaped back to (B, E, N, D)

Sharding: expert parallelism — core e runs expert e on its contiguous
token block xe[e] (4096 tokens).  No collectives needed.

Per-core layout: hidden activations kept transposed ("hT" = [f, tok]) so both
weight matrices are consumed in their NATIVE layouts:
  GEMM1: hT[f,tok]  = (w1[d,f] as lhsT).T @ xT[d,tok]
  GEMM2: out[tok,d] = (hT[f,tok] slice as lhsT).T @ w2[f,d]

Key profiling fact this schedule is built around: DMA packets that cast
fp32->bf16 in flight run at roughly HALF the DMA engine byte rate (~150 GB/s
aggregate vs ~280 GB/s for raw copies).  The 984us baseline streamed all of
w1+w2 through cast-DMAs on one SWDGE queue, so weights took ~210us to arrive
and GEMM2 of chunk 0 stalled ~44us.  Here:
  - w1/w2 are DMA'd as RAW fp32 quarter-groups (1MB each) into small SBUF
    staging tiles — w1 g0-g6 + w2 g3-g7 on the scalar HWDGE queue, w1 g7 +
    w2 g0-g2 on the sync HWDGE queue (both queues run at full rate in
    parallel) — and cast to resident bf16 tiles by the otherwise-idle
    Vector engine (~35us of DVE time, hidden behind the PE).
  - GEMM2 of chunk 0 is emitted f-group-major (all 8 PSUM tiles accumulate
    per group) so it consumes w2 groups at the rate they stream in.
  - x chunk 0: fp32 per-tm quarter loads on sync + PE-mode fp32 transpose
    (PE is idle until w1 g0 lands anyway); chunks 1-7: fp32->bf16 cast-DMA
    to DRAM scratch on SWDGE (which now carries nothing else) + XBAR
    DMA-transposes on sync, prefetched one chunk ahead.
  - single 8-buf PSUM pool shared by transposes/GEMM1/GEMM2.

Compute dtype bf16 (fp32 PSUM accumulation), gelu on ScalarE matching
jax.nn.gelu(approximate=True): end-to-end rel-err ~3.4e-3.
"""

import numpy as np

import concourse.bacc as bacc
import concourse.mybir as mybir
import concourse.tile as tile
from concourse.bass_utils import run_bass_kernel_spmd

E, B, N, D, F = 8, 2, 2048, 1024, 4096
TOK = B * N            # tokens per expert / per core
TC = 512               # token chunk processed per pipeline stage
NCHUNK = TOK // TC     # 8
P = 128
DO = D // P            # 8  d-tiles (GEMM1 contraction)
FO = F // P            # 32 f-tiles (GEMM2 contraction)
FG = 8                 # weight f-groups of 512 (4 f-tiles each)

F32 = mybir.dt.float32
BF16 = mybir.dt.bfloat16
GELU = mybir.ActivationFunctionType.Gelu_apprx_tanh


def _build_kernel(tc_ctx, nc, x, w1, b1, w2, b2, out):
    with (
        tc_ctx.tile_pool(name="wpool", bufs=1) as wp,
        tc_ctx.tile_pool(name="sscalar", bufs=3) as stg_a,
        tc_ctx.tile_pool(name="ssync", bufs=2) as stg_s,
        tc_ctx.tile_pool(name="xpool", bufs=2) as xp,
        tc_ctx.tile_pool(name="hpool", bufs=1) as hp,
        tc_ctx.tile_pool(name="opool", bufs=3) as op,
        tc_ctx.tile_pool(name="cpool", bufs=1) as cp,
        tc_ctx.tile_pool(name="dram", bufs=1, space="DRAM") as dp,
        tc_ctx.tile_pool(name="ps", bufs=8, space="PSUM") as psp,
    ):
        # w1 tile (ki, do, fj) = w1[do*128+ki, fg*512+fj] : lhsT for GEMM1
        w1r = w1.rearrange("(do ki) f -> ki do f", ki=P)
        # w2 tile (ki, m, dj) = w2[fg*512 + m*128 + ki, dj] : rhs for GEMM2
        w2r = w2.rearrange("(fg m ki) d -> ki fg m d", ki=P, m=4)
        w1g = [
            wp.tile([P, DO, 512], BF16, tag=f"w1g{fg}", name=f"w1g{fg}")
            for fg in range(FG)
        ]
        w2g = [
            wp.tile([P, 4, D], BF16, tag=f"w2g{fg}", name=f"w2g{fg}")
            for fg in range(FG)
        ]

        # ---- weight load plan: raw fp32 quarters -> staging -> vector cast.
        # Each quarter is [128, 1024] f32 (1MB).  DMA emission below; the
        # vector casts are emitted in consumption order further down.
        def w1_quarter_dma(eng, pool, g, dq):
            s = pool.tile([P, 1024], F32, tag="sq", name=f"w1s{g}_{dq}")
            eng.dma_start(
                s.rearrange("p (do f) -> p do f", do=2),
                w1r[:, 2 * dq:2 * dq + 2, g * 512:(g + 1) * 512],
            )
            return s

        def w2_quarter_dma(eng, pool, g, m):
            s = pool.tile([P, 1024], F32, tag="sq", name=f"w2s{g}_{m}")
            eng.dma_start(s, w2r[:, g, m, :])
            return s

        # scalar HWDGE queue: w1 g0..g6, then w2 g3..g7
        w1s = {}
        w2s = {}
        for g in range(7):
            for dq in range(4):
                w1s[(g, dq)] = w1_quarter_dma(nc.scalar, stg_a, g, dq)
        for g in range(3, 8):
            for m in range(4):
                w2s[(g, m)] = w2_quarter_dma(nc.scalar, stg_a, g, m)

        # x chunks: fp32->bf16 cast into DRAM scratch on SWDGE (which
        # carries nothing else).  Chunk 0 goes first as 4 fine quarters so
        # its XBAR transposes start after ~0.5MB instead of the full chunk.
        xb = [None] * NCHUNK
        xb[0] = [None] * 4
        for q in range(4):
            t = dp.tile([TC, D // 4], BF16, tag=f"xb0_{q}")
            nc.gpsimd.dma_start(
                t, x[0:TC, q * (D // 4):(q + 1) * (D // 4)]
            )
            xb[0][q] = t

        # sync HWDGE queue: chunk-0 XBAR transposes, biases, then
        # w1 g7 + w2 g0..g2 quarters.
        xT0 = xp.tile([P, DO, TC], BF16, tag="xT")
        for do in range(DO):
            src = xb[0][do // 2]
            nc.sync.dma_start_transpose(
                xT0[:, do, :], src[:, (do % 2) * P:(do % 2 + 1) * P]
            )
        # b1 on partitions (f-inner), one column per f-tile -> activation bias
        b1sb = cp.tile([P, FO], F32, tag="b1")
        nc.sync.dma_start(b1sb, b1.rearrange("(fo fi) -> fi fo", fi=P))
        # b2 replicated across all 128 partitions (free dim = d)
        b2sb = cp.tile([P, D], BF16, tag="b2")
        nc.gpsimd.dma_start(b2sb[0:1, :], b2[None, :])
        k = 1
        while k < P:
            nc.sync.dma_start(b2sb[k:2 * k, :], b2sb[0:k, :])
            k *= 2

        for c in range(1, NCHUNK):
            xb[c] = [None, None]
            for half in range(2):
                t = dp.tile([TC, D // 2], BF16, tag=f"xb{c}_{half}")
                nc.gpsimd.dma_start(
                    t, x[c * TC:(c + 1) * TC, half * (D // 2):(half + 1) * (D // 2)]
                )
                xb[c][half] = t

        for dq in range(4):
            w1s[(7, dq)] = w1_quarter_dma(nc.sync, stg_s, 7, dq)
        for g in range(3):
            for m in range(4):
                w2s[(g, m)] = w2_quarter_dma(nc.sync, stg_s, g, m)


        # ---- weight casts on vector, in arrival/consumption order ----
        def cast_w1(g):
            for dq in range(4):
                nc.vector.tensor_copy(
                    w1g[g][:, 2 * dq:2 * dq + 2, :],
                    w1s[(g, dq)].rearrange("p (do f) -> p do f", do=2),
                )

        def cast_w2(g):
            for m in range(4):
                nc.vector.tensor_copy(w2g[g][:, m, :], w2s[(g, m)])

        for g in (0, 1, 7, 2, 3):
            cast_w1(g)
        cast_w2(0)
        cast_w1(4)
        cast_w2(1)
        cast_w1(5)
        cast_w2(2)
        cast_w1(6)
        for g in range(3, 8):
            cast_w2(g)

        xTs = {0: xT0}

        # ---- main pipeline over token chunks ----
        for c in range(NCHUNK):
            # prefetch transpose of chunk c+1 (sync queue, before this
            # chunk's output stores)
            if c + 1 < NCHUNK:
                nxt = xp.tile([P, DO, TC], BF16, tag="xT")
                for do in range(DO):
                    src = xb[c + 1][do // 4]
                    nc.sync.dma_start_transpose(
                        nxt[:, do, :], src[:, (do % 4) * P:(do % 4 + 1) * P]
                    )
                xTs[c + 1] = nxt
            xT = xTs.pop(c)

            # GEMM1 + bias + gelu -> hT[f-part, fo, tok] (bf16)
            hT = hp.tile([P, FO, TC], BF16, tag="hT")
            for fo in range(FO):
                ps = psp.tile([P, TC], F32, tag="ps")
                w1t = w1g[fo // 4]
                fi = (fo % 4) * P
                for do in range(DO):
                    nc.tensor.matmul(
                        ps,
                        w1t[:, do, fi:fi + P],
                        xT[:, do, :],
                        start=(do == 0),
                        stop=(do == DO - 1),
                    )
                nc.scalar.activation(
                    hT[:, fo, :], ps, GELU, bias=b1sb[:, fo:fo + 1]
                )

            # GEMM2 + bias -> out[tok, d] natural layout
            if c == 0:
                # f-group-major: all 8 psum tiles accumulate per group, so
                # matmuls consume w2 groups as they stream in.
                pts = [
                    psp.tile([P, 512], F32, tag="ps", name=f"ps2_{i}")
                    for i in range(8)
                ]
                for g in range(FG):
                    for tt in range(TC // P):
                        for dh in range(2):
                            pt = pts[tt * 2 + dh]
                            for j in range(4):
                                fo = g * 4 + j
                                nc.tensor.matmul(
                                    pt,
                                    hT[:, fo, tt * P:(tt + 1) * P],
                                    w2g[g][:, j, dh * 512:(dh + 1) * 512],
                                    start=(fo == 0),
                                    stop=(fo == FO - 1),
                                )
                for tt in range(TC // P):
                    for dh in range(2):
                        pt = pts[tt * 2 + dh]
                        osb = op.tile([P, 512], F32, tag="osb")
                        nc.vector.tensor_tensor(
                            osb, pt, b2sb[:, dh * 512:(dh + 1) * 512],
                            mybir.AluOpType.add,
                        )
                        row0 = c * TC + tt * P
                        nc.sync.dma_start(
                            out[row0:row0 + P, dh * 512:(dh + 1) * 512], osb
                        )
            else:
                for tt in range(TC // P):
                    for dh in range(2):
                        ps2t = psp.tile([P, 512], F32, tag="ps")
                        for fo in range(FO):
                            nc.tensor.matmul(
                                ps2t,
                                hT[:, fo, tt * P:(tt + 1) * P],
                                w2g[fo // 4][:, fo % 4, dh * 512:(dh + 1) * 512],
                                start=(fo == 0),
                                stop=(fo == FO - 1),
                            )
                        osb = op.tile([P, 512], F32, tag="osb")
                        nc.vector.tensor_tensor(
                            osb, ps2t, b2sb[:, dh * 512:(dh + 1) * 512],
                            mybir.AluOpType.add,
                        )
                        row0 = c * TC + tt * P
                        nc.sync.dma_start(
                            out[row0:row0 + P, dh * 512:(dh + 1) * 512], osb
                        )


_NC_CACHE = None


def _get_nc():
    global _NC_CACHE
    if _NC_CACHE is None:
        nc = bacc.Bacc(
            "TRN2", target_bir_lowering=False, num_devices=E, num_swdge_queues=4
        )
        x = nc.dram_tensor("x", [TOK, D], F32, kind="ExternalInput").ap()
        w1 = nc.dram_tensor("w1", [D, F], F32, kind="ExternalInput").ap()
        b1 = nc.dram_tensor("b1", [F], F32, kind="ExternalInput").ap()
        w2 = nc.dram_tensor("w2", [F, D], F32, kind="ExternalInput").ap()
        b2 = nc.dram_tensor("b2", [D], F32, kind="ExternalInput").ap()
        out = nc.dram_tensor("out", [TOK, D], F32, kind="ExternalOutput").ap()
        with tile.TileContext(nc) as tctx:
            _build_kernel(tctx, nc, x, w1, b1, w2, b2, out)
        nc.compile()
        _NC_CACHE = nc
    return _NC_CACHE


def kernel(run_opts=None, **inputs):
    x = np.ascontiguousarray(inputs["x"], dtype=np.float32)
    w1 = np.ascontiguousarray(inputs["w1"], dtype=np.float32)
    b1 = np.ascontiguousarray(inputs["b1"], dtype=np.float32)
    w2 = np.ascontiguousarray(inputs["w2"], dtype=np.float32)
    b2 = np.ascontiguousarray(inputs["b2"], dtype=np.float32)

    # x.view(E, B, N, D) in the reference is a pure reshape: expert e owns the
    # contiguous token block e of the flattened (E*B*N, D) buffer.
    xf = x.reshape(E, TOK, D)
    in_maps = [
        {"x": xf[e], "w1": w1[e], "b1": b1[e], "w2": w2[e], "b2": b2[e]}
        for e in range(E)
    ]
    nc = _get_nc()
    res = run_bass_kernel_spmd(
        nc, in_maps, core_ids=list(range(E)), **(run_opts or {})
    )
    outs = np.stack([res.results[e]["out"] for e in range(E)])  # (E, TOK, D)
    if run_opts:
        kernel.last_results = res
    # outputs.view(B, E, N, D) in the reference: reinterpret (E, B*N, D) buffer
    return outs.reshape(B, E, N, D)


# revision 13
# speedup vs baseline: 1.2459x; 1.2459x over previous
"""MoE grouped-GEMM expert MLP for Trainium2, expert-parallel over 8 NeuronCores.

Problem: x:(B=2, E=8, N=2048, D=1024), per-expert 2-layer GELU MLP with
w1:(E, D, F=4096), w2:(E, F, D).  Reference computes
  xe = x.reshape(E, B*N, D)          # pure buffer reinterpretation
  h  = gelu_tanh(xe @ w1 + b1)
  out= h @ w2 + b2                   # reshaped back to (B, E, N, D)

Sharding: expert parallelism — core e runs expert e on its contiguous
token block xe[e] (4096 tokens).  No collectives needed.

Per-core layout: hidden activations kept transposed ("hT" = [f, tok]) so both
weight matrices are consumed in their NATIVE layouts:
  GEMM1: hT[f,tok]  = (w1[d,f] as lhsT).T @ xT[d,tok]
  GEMM2: out[tok,d] = (hT[f,tok] slice as lhsT).T @ w2[f,d]

Profiling facts this schedule is built around (from NTFF DMA packet data):
  - DMA packets that cast fp32->bf16 in flight run at roughly HALF the DMA
    engine byte rate (~150 GB/s aggregate vs ~280 GB/s raw).
  - XBAR DMA-transposes are much slower still (~10-13 GB/s per transfer):
    one token-chunk's transposes are ~100us of queue time, which is what
    actually paced the 984us baseline (8 transposes/chunk on one queue).
Consequences:
  - No DMA transposes at all.  Every chunk's xT is produced by PE-mode
    transposes (bf16, 1 cycle/row: ~1.7us/chunk of PE time).  x streams in
    as fp32->bf16 cast-DMAs on SWDGE into a small SBUF staging pool
    (chunk 0: raw fp32 on the sync queue + fp32 PE transpose, to start the
    PE ~10us in instead of waiting ~15us for a cast).
  - w1/w2 are DMA'd as RAW fp32 slices into small staging tiles — w1 g0-g5
    + w2 g3-g7 as eighth-groups on the scalar HWDGE queue, w1 g6-g7 +
    w2 g0-g2 as quarter-groups on the sync queue — and cast to resident
    bf16 tiles by the otherwise-idle Vector engine.
  - GEMM2 of chunk 0 is emitted f-group-major (all 8 PSUM tiles accumulate
    per group) so it consumes w2 groups at the rate they stream in.
  - single 8-buf PSUM pool shared by transposes/GEMM1/GEMM2; single xT
    buffer (the PE's program order makes chunk c+1's transposes wait for
    GEMM1 of chunk c, which is exactly when xT is dead anyway).

Compute dtype bf16 (fp32 PSUM accumulation), gelu on ScalarE matching
jax.nn.gelu(approximate=True): end-to-end rel-err ~3.4e-3.
"""

import numpy as np

import concourse.bacc as bacc
import concourse.mybir as mybir
import concourse.tile as tile
from concourse.bass_utils import run_bass_kernel_spmd
from concourse.masks import make_identity

E, B, N, D, F = 8, 2, 2048, 1024, 4096
TOK = B * N            # tokens per expert / per core
TC = 512               # token chunk processed per pipeline stage
NCHUNK = TOK // TC     # 8
P = 128
DO = D // P            # 8  d-tiles (GEMM1 contraction)
FO = F // P            # 32 f-tiles (GEMM2 contraction)
FG = 8                 # weight f-groups of 512 (4 f-tiles each)

F32 = mybir.dt.float32
BF16 = mybir.dt.bfloat16
GELU = mybir.ActivationFunctionType.Gelu_apprx_tanh


def _build_kernel(tc_ctx, nc, x, w1, b1, w2, b2, out):
    with (
        tc_ctx.tile_pool(name="wpool", bufs=1) as wp,
        tc_ctx.tile_pool(name="sscalar", bufs=4) as stg_a,
        tc_ctx.tile_pool(name="ssync", bufs=2) as stg_s,
        tc_ctx.tile_pool(name="xpool", bufs=1) as xp,
        tc_ctx.tile_pool(name="xbpool", bufs=2) as xbp,
        tc_ctx.tile_pool(name="hpool", bufs=1) as hp,
        tc_ctx.tile_pool(name="opool", bufs=2) as op,
        tc_ctx.tile_pool(name="cpool", bufs=1) as cp,
        tc_ctx.tile_pool(name="ps", bufs=8, space="PSUM") as psp,
    ):
        # identity for PE-mode transposes
        ident = cp.tile([P, P], F32, tag="ident")
        make_identity(nc, ident)
        identb = cp.tile([P, P], BF16, tag="identb")
        nc.vector.tensor_copy(identb, ident)

        # w1 tile (ki, do, fj) = w1[do*128+ki, fg*512+fj] : lhsT for GEMM1
        w1r = w1.rearrange("(do ki) f -> ki do f", ki=P)
        # w2 tile (ki, m, dj) = w2[fg*512 + m*128 + ki, dj] : rhs for GEMM2
        w2r = w2.rearrange("(fg m ki) d -> ki fg m d", ki=P, m=4)
        w1g = [
            wp.tile([P, DO, 512], BF16, tag=f"w1g{fg}", name=f"w1g{fg}")
            for fg in range(FG)
        ]
        w2g = [
            wp.tile([P, 4, D], BF16, tag=f"w2g{fg}", name=f"w2g{fg}")
            for fg in range(FG)
        ]

        # ---- weight loads: raw fp32 -> staging -> vector cast to bf16 ----
        # scalar HWDGE queue: w1 g0..g5 + w2 g3..g7 as [128, 512] eighths.
        # sync HWDGE queue:   w1 g6,g7 + w2 g0..g2 as [128, 1024] quarters
        # (after the chunk-0 x fp32 loads below).
        w1e = {}
        w2e = {}
        for g in range(6):
            for e8 in range(8):
                s = stg_a.tile([P, 512], F32, tag="s8", name=f"w1s{g}_{e8}")
                nc.scalar.dma_start(
                    s, w1r[:, e8, g * 512:(g + 1) * 512]
                )
                w1e[(g, e8)] = s
        for g in range(3, 8):
            for m in range(4):
                for h2 in range(2):
                    s = stg_a.tile([P, 512], F32, tag="s8", name=f"w2s{g}_{m}_{h2}")
                    nc.scalar.dma_start(
                        s, w2r[:, g, m, h2 * 512:(h2 + 1) * 512]
                    )
                    w2e[(g, m, h2)] = s

        # x chunks 1-7: fp32->bf16 cast-DMA straight into SBUF halves on
        # SWDGE (256 tokens per half), which carries nothing else.
        xh = [[None, None] for _ in range(NCHUNK)]
        for c in range(1, NCHUNK):
            for h in range(2):
                t = xbp.tile([P, 2, D], BF16, tag="xh", name=f"xh{c}_{h}")
                nc.gpsimd.dma_start(
                    t,
                    x[c * TC + h * 256:c * TC + (h + 1) * 256, :].rearrange(
                        "(tm p) d -> p tm d", p=P
                    ),
                )
                xh[c][h] = t

        # sync HWDGE queue: chunk-0 x fp32 quarters (128 tokens each), b1,
        # then the sync-side weight quarters, then b2.
        xq = []
        for tm in range(4):
            s = stg_s.tile([P, 1024], F32, tag="s4", name=f"xq{tm}")
            nc.sync.dma_start(s, x[tm * P:(tm + 1) * P, :])
            xq.append(s)

        # b1 on partitions (f-inner), one column per f-tile -> activation bias
        b1sb = cp.tile([P, FO], F32, tag="b1")
        nc.sync.dma_start(b1sb, b1.rearrange("(fo fi) -> fi fo", fi=P))

        w1q = {}
        w2q = {}
        for g in (6, 7):
            for q in range(4):
                s = stg_s.tile([P, 1024], F32, tag="s4", name=f"w1q{g}_{q}")
                nc.sync.dma_start(
                    s.rearrange("p (do f) -> p do f", do=2),
                    w1r[:, 2 * q:2 * q + 2, g * 512:(g + 1) * 512],
                )
                w1q[(g, q)] = s
        for g in range(3):
            for m in range(4):
                s = stg_s.tile([P, 1024], F32, tag="s4", name=f"w2q{g}_{m}")
                nc.sync.dma_start(s, w2r[:, g, m, :])
                w2q[(g, m)] = s

        # b2 replicated across all 128 partitions (free dim = d); the
        # doubling chain sits late on the sync queue so it can't head-block
        # the weight quarters.
        b2sb = cp.tile([P, D], F32, tag="b2")
        nc.sync.dma_start(b2sb[0:1, :], b2[None, :])
        k = 1
        while k < P:
            nc.sync.dma_start(b2sb[k:2 * k, :], b2sb[0:k, :])
            k *= 2

        # ---- chunk 0 transpose on PE (fp32 in, bf16 out via vector) ----
        xT = xp.tile([P, DO, TC], BF16, tag="xT")
        for tm in range(4):
            for dg in range(2):
                pt = psp.tile([P, 4, P], F32, tag="ps", name=f"psT{tm}_{dg}")
                for dj in range(4):
                    do = dg * 4 + dj
                    nc.tensor.transpose(
                        pt[:, dj, :],
                        xq[tm][:, do * P:(do + 1) * P],
                        ident,
                    )
                nc.vector.tensor_copy(
                    xT[:, dg * 4:(dg + 1) * 4, tm * P:(tm + 1) * P], pt
                )

        # ---- weight casts on vector, in arrival/consumption order ----
        def cast_w1_e8(g):
            for e8 in range(8):
                nc.vector.tensor_copy(w1g[g][:, e8, :], w1e[(g, e8)])

        def cast_w1_q(g):
            for q in range(4):
                nc.vector.tensor_copy(
                    w1g[g][:, 2 * q:2 * q + 2, :],
                    w1q[(g, q)].rearrange("p (do f) -> p do f", do=2),
                )

        def cast_w2_q(g):
            for m in range(4):
                nc.vector.tensor_copy(w2g[g][:, m, :], w2q[(g, m)])

        def cast_w2_e8(g):
            for m in range(4):
                for h2 in range(2):
                    nc.vector.tensor_copy(
                        w2g[g][:, m, h2 * 512:(h2 + 1) * 512],
                        w2e[(g, m, h2)],
                    )

        for g in range(6):
            cast_w1_e8(g)
        cast_w1_q(6)
        cast_w1_q(7)
        for g in range(3):
            cast_w2_q(g)
        for g in range(3, 8):
            cast_w2_e8(g)

        # ---- main pipeline over token chunks ----
        for c in range(NCHUNK):
            # GEMM1 + bias + gelu -> hT[f-part, fo, tok] (bf16)
            hT = hp.tile([P, FO, TC], BF16, tag="hT")
            for fo in range(FO):
                ps = psp.tile([P, TC], F32, tag="ps")
                w1t = w1g[fo // 4]
                fi = (fo % 4) * P
                for do in range(DO):
                    nc.tensor.matmul(
                        ps,
                        w1t[:, do, fi:fi + P],
                        xT[:, do, :],
                        start=(do == 0),
                        stop=(do == DO - 1),
                    )
                nc.scalar.activation(
                    hT[:, fo, :], ps, GELU, bias=b1sb[:, fo:fo + 1]
                )

            # chunk c+1 transposes on PE (bf16): xT is dead now (GEMM1 of
            # chunk c was its last reader, and the PE runs in order).
            if c + 1 < NCHUNK:
                for h in range(2):
                    src = xh[c + 1][h]
                    for dg in range(2):
                        pt = psp.tile(
                            [P, 4, 256], BF16, tag="ps", name=f"ptb{h}_{dg}"
                        )
                        for dj in range(4):
                            do = dg * 4 + dj
                            for tm in range(2):
                                nc.tensor.transpose(
                                    pt[:, dj, tm * P:(tm + 1) * P],
                                    src[:, tm, do * P:(do + 1) * P],
                                    identb,
                                )
                        nc.scalar.activation(
                            xT[:, dg * 4:(dg + 1) * 4, h * 256:(h + 1) * 256],
                            pt,
                            mybir.ActivationFunctionType.Copy,
                        )

            # GEMM2 + bias -> out[tok, d] natural layout
            if c == 0:
                # f-group-major: all 8 psum tiles accumulate per group, so
                # matmuls consume w2 groups as they stream in.
                pts = [
                    psp.tile([P, 512], F32, tag="ps", name=f"ps2_{i}")
                    for i in range(8)
                ]
                for g in range(FG):
                    for tt in range(TC // P):
                        for dh in range(2):
                            pt = pts[tt * 2 + dh]
                            for j in range(4):
                                fo = g * 4 + j
                                nc.tensor.matmul(
                                    pt,
                                    hT[:, fo, tt * P:(tt + 1) * P],
                                    w2g[g][:, j, dh * 512:(dh + 1) * 512],
                                    start=(fo == 0),
                                    stop=(fo == FO - 1),
                                )
                for tt in range(TC // P):
                    for dh in range(2):
                        pt = pts[tt * 2 + dh]
                        osb = op.tile([P, 512], F32, tag="osb")
                        nc.vector.tensor_tensor(
                            osb, pt, b2sb[:, dh * 512:(dh + 1) * 512],
                            mybir.AluOpType.add,
                        )
                        row0 = c * TC + tt * P
                        nc.sync.dma_start(
                            out[row0:row0 + P, dh * 512:(dh + 1) * 512], osb
                        )
            else:
                for tt in range(TC // P):
                    for dh in range(2):
                        ps2t = psp.tile([P, 512], F32, tag="ps")
                        for fo in range(FO):
                            nc.tensor.matmul(
                                ps2t,
                                hT[:, fo, tt * P:(tt + 1) * P],
                                w2g[fo // 4][:, fo % 4, dh * 512:(dh + 1) * 512],
                                start=(fo == 0),
                                stop=(fo == FO - 1),
                            )
                        osb = op.tile([P, 512], F32, tag="osb")
                        nc.vector.tensor_tensor(
                            osb, ps2t, b2sb[:, dh * 512:(dh + 1) * 512],
                            mybir.AluOpType.add,
                        )
                        row0 = c * TC + tt * P
                        nc.sync.dma_start(
                            out[row0:row0 + P, dh * 512:(dh + 1) * 512], osb
                        )


_NC_CACHE = None


def _get_nc():
    global _NC_CACHE
    if _NC_CACHE is None:
        nc = bacc.Bacc(
            "TRN2", target_bir_lowering=False, num_devices=E, num_swdge_queues=4
        )
        x = nc.dram_tensor("x", [TOK, D], F32, kind="ExternalInput").ap()
        w1 = nc.dram_tensor("w1", [D, F], F32, kind="ExternalInput").ap()
        b1 = nc.dram_tensor("b1", [F], F32, kind="ExternalInput").ap()
        w2 = nc.dram_tensor("w2", [F, D], F32, kind="ExternalInput").ap()
        b2 = nc.dram_tensor("b2", [D], F32, kind="ExternalInput").ap()
        out = nc.dram_tensor("out", [TOK, D], F32, kind="ExternalOutput").ap()
        with tile.TileContext(nc) as tctx:
            _build_kernel(tctx, nc, x, w1, b1, w2, b2, out)
        nc.compile()
        _NC_CACHE = nc
    return _NC_CACHE


def kernel(run_opts=None, **inputs):
    x = np.ascontiguousarray(inputs["x"], dtype=np.float32)
    w1 = np.ascontiguousarray(inputs["w1"], dtype=np.float32)
    b1 = np.ascontiguousarray(inputs["b1"], dtype=np.float32)
    w2 = np.ascontiguousarray(inputs["w2"], dtype=np.float32)
    b2 = np.ascontiguousarray(inputs["b2"], dtype=np.float32)

    # x.view(E, B, N, D) in the reference is a pure reshape: expert e owns the
    # contiguous token block e of the flattened (E*B*N, D) buffer.
    xf = x.reshape(E, TOK, D)
    in_maps = [
        {"x": xf[e], "w1": w1[e], "b1": b1[e], "w2": w2[e], "b2": b2[e]}
        for e in range(E)
    ]
    nc = _get_nc()
    res = run_bass_kernel_spmd(
        nc, in_maps, core_ids=list(range(E)), **(run_opts or {})
    )
    outs = np.stack([res.results[e]["out"] for e in range(E)])  # (E, TOK, D)
    if run_opts:
        kernel.last_results = res
    # outputs.view(B, E, N, D) in the reference: reinterpret (E, B*N, D) buffer
    return outs.reshape(B, E, N, D)


# revision 15
# speedup vs baseline: 1.2713x; 1.0204x over previous
"""MoE grouped-GEMM expert MLP for Trainium2, expert-parallel over 8 NeuronCores.

Problem: x:(B=2, E=8, N=2048, D=1024), per-expert 2-layer GELU MLP with
w1:(E, D, F=4096), w2:(E, F, D).  Reference computes
  xe = x.reshape(E, B*N, D)          # pure buffer reinterpretation
  h  = gelu_tanh(xe @ w1 + b1)
  out= h @ w2 + b2                   # reshaped back to (B, E, N, D)

Sharding: expert parallelism — core e runs expert e on its contiguous
token block xe[e] (4096 tokens).  No collectives needed.

Per-core layout: hidden activations kept transposed ("hT" = [f, tok]) so both
weight matrices are consumed in their NATIVE layouts:
  GEMM1: hT[f,tok]  = (w1[d,f] as lhsT).T @ xT[d,tok]
  GEMM2: out[tok,d] = (hT[f,tok] slice as lhsT).T @ w2[f,d]

Profiling facts this schedule is built around (from NTFF DMA packet data):
  - DMA packets that cast fp32->bf16 in flight run at roughly HALF the DMA
    engine byte rate (~150 GB/s aggregate vs ~280 GB/s raw).
  - XBAR DMA-transposes are much slower still (~10-13 GB/s per transfer):
    one token-chunk's transposes are ~100us of queue time, which is what
    actually paced the 984us baseline (8 transposes/chunk on one queue).
Consequences:
  - No DMA transposes at all.  Every chunk's xT is produced by PE-mode
    transposes (bf16, 1 cycle/row: ~1.7us/chunk of PE time).  x streams in
    as fp32->bf16 cast-DMAs on SWDGE into a small SBUF staging pool
    (chunk 0: raw fp32 on the sync queue + fp32 PE transpose, to start the
    PE ~10us in instead of waiting ~15us for a cast).
  - w1/w2 are DMA'd as RAW fp32 slices into small staging tiles — w1 g0-g5
    + w2 g3-g7 as eighth-groups on the scalar HWDGE queue, w1 g6-g7 +
    w2 g0-g2 as quarter-groups on the sync queue — and cast to resident
    bf16 tiles by the otherwise-idle Vector engine.
  - GEMM2 of chunk 0 is emitted f-group-major (all 8 PSUM tiles accumulate
    per group) so it consumes w2 groups at the rate they stream in.
  - single 8-buf PSUM pool shared by transposes/GEMM1/GEMM2; single xT
    buffer (the PE's program order makes chunk c+1's transposes wait for
    GEMM1 of chunk c, which is exactly when xT is dead anyway).

Compute dtype bf16 (fp32 PSUM accumulation), gelu on ScalarE matching
jax.nn.gelu(approximate=True): end-to-end rel-err ~3.4e-3.
"""

import numpy as np

import concourse.bacc as bacc
import concourse.mybir as mybir
import concourse.tile as tile
from concourse.bass_utils import run_bass_kernel_spmd
from concourse.masks import make_identity

E, B, N, D, F = 8, 2, 2048, 1024, 4096
TOK = B * N            # tokens per expert / per core
TC = 512               # token chunk processed per pipeline stage
NCHUNK = TOK // TC     # 8
P = 128
DO = D // P            # 8  d-tiles (GEMM1 contraction)
FO = F // P            # 32 f-tiles (GEMM2 contraction)
FG = 8                 # weight f-groups of 512 (4 f-tiles each)

F32 = mybir.dt.float32
BF16 = mybir.dt.bfloat16
GELU = mybir.ActivationFunctionType.Gelu_apprx_tanh


def _build_kernel(tc_ctx, nc, x, w1, b1, w2, b2, out):
    with (
        tc_ctx.tile_pool(name="wpool", bufs=1) as wp,
        tc_ctx.tile_pool(name="sscalar", bufs=4) as stg_a,
        tc_ctx.tile_pool(name="ssync", bufs=2) as stg_s,
        tc_ctx.tile_pool(name="xpool", bufs=1) as xp,
        tc_ctx.tile_pool(name="xbpool", bufs=2) as xbp,
        tc_ctx.tile_pool(name="hpool", bufs=1) as hp,
        tc_ctx.tile_pool(name="opool", bufs=2) as op,
        tc_ctx.tile_pool(name="cpool", bufs=1) as cp,
        tc_ctx.tile_pool(name="ps", bufs=8, space="PSUM") as psp,
    ):
        # identity for PE-mode transposes
        ident = cp.tile([P, P], F32, tag="ident")
        make_identity(nc, ident)
        identb = cp.tile([P, P], BF16, tag="identb")
        nc.vector.tensor_copy(identb, ident)

        # w1 tile (ki, do, fj) = w1[do*128+ki, fg*512+fj] : lhsT for GEMM1
        w1r = w1.rearrange("(do ki) f -> ki do f", ki=P)
        # w2 tile (ki, m, dj) = w2[fg*512 + m*128 + ki, dj] : rhs for GEMM2
        w2r = w2.rearrange("(fg m ki) d -> ki fg m d", ki=P, m=4)
        w1g = [
            wp.tile([P, DO, 512], BF16, tag=f"w1g{fg}", name=f"w1g{fg}")
            for fg in range(FG)
        ]
        w2g = [
            wp.tile([P, 4, D], BF16, tag=f"w2g{fg}", name=f"w2g{fg}")
            for fg in range(FG)
        ]

        # ---- weight loads: raw fp32 -> staging -> vector cast to bf16 ----
        # scalar HWDGE queue: w1 g0..g5 + w2 g3..g7 as [128, 512] eighths.
        # sync HWDGE queue:   w1 g6,g7 + w2 g0..g2 as [128, 1024] quarters
        # (after the chunk-0 x fp32 loads below).
        w1e = {}
        w2e = {}

        def w1_e8_dma(eng, pool, g, e8):
            t = pool.tile([P, 512], F32, tag="s8", name=f"w1s{g}_{e8}")
            eng.dma_start(t, w1r[:, e8, g * 512:(g + 1) * 512])
            w1e[(g, e8)] = t

        def w2_e8_dma(eng, pool, g, m, h2):
            t = pool.tile([P, 512], F32, tag="s8", name=f"w2s{g}_{m}_{h2}")
            eng.dma_start(t, w2r[:, g, m, h2 * 512:(h2 + 1) * 512])
            w2e[(g, m, h2)] = t

        for g in range(4):
            for e8 in range(8):
                w1_e8_dma(nc.scalar, stg_a, g, e8)
        for g in range(3, 8):
            for m in range(4):
                for h2 in range(2):
                    w2_e8_dma(nc.scalar, stg_a, g, m, h2)

        # x chunks 1-7: fp32->bf16 cast-DMA straight into SBUF halves on
        # SWDGE (256 tokens per half), which carries nothing else.
        xh = [[None, None] for _ in range(NCHUNK)]
        for c in range(1, NCHUNK):
            for h in range(2):
                t = xbp.tile([P, 2, D], BF16, tag="xh", name=f"xh{c}_{h}")
                nc.gpsimd.dma_start(
                    t,
                    x[c * TC + h * 256:c * TC + (h + 1) * 256, :].rearrange(
                        "(tm p) d -> p tm d", p=P
                    ),
                )
                xh[c][h] = t

        # sync HWDGE queue: chunk-0 x fp32 quarters (128 tokens each), b1,
        # then the sync-side weight quarters, then b2.
        xq = {}
        for tm in range(4):
            for dh in range(2):
                t = stg_s.tile([P, 512], F32, tag="s8", name=f"xq{tm}_{dh}")
                nc.sync.dma_start(
                    t, x[tm * P:(tm + 1) * P, dh * 512:(dh + 1) * 512]
                )
                xq[(tm, dh)] = t

        # b1 on partitions (f-inner), one column per f-tile -> activation bias
        b1sb = cp.tile([P, FO], F32, tag="b1")
        nc.sync.dma_start(b1sb, b1.rearrange("(fo fi) -> fi fo", fi=P))

        for g in range(4, 8):
            for e8 in range(8):
                w1_e8_dma(nc.sync, stg_s, g, e8)
        for g in range(3):
            for m in range(4):
                for h2 in range(2):
                    w2_e8_dma(nc.sync, stg_s, g, m, h2)

        # b2 replicated across all 128 partitions (free dim = d); the
        # doubling chain sits late on the sync queue so it can't head-block
        # the weight quarters.
        b2sb = cp.tile([P, D], F32, tag="b2")
        nc.sync.dma_start(b2sb[0:1, :], b2[None, :])
        k = 1
        while k < P:
            nc.sync.dma_start(b2sb[k:2 * k, :], b2sb[0:k, :])
            k *= 2

        # ---- weight casts on vector: w1 g0 first (GEMM1 needs it ~10us
        # in; the chunk-0 copies below land on vector right after) ----
        def cast_w1_e8(g):
            for e8 in range(8):
                nc.vector.tensor_copy(w1g[g][:, e8, :], w1e[(g, e8)])

        cast_w1_e8(0)

        # ---- chunk 0 transpose on PE (fp32 in, bf16 out via vector) ----
        xT = xp.tile([P, DO, TC], BF16, tag="xT")
        for tm in range(4):
            for dg in range(2):
                pt = psp.tile([P, 4, P], F32, tag="ps", name=f"psT{tm}_{dg}")
                for dj in range(4):
                    nc.tensor.transpose(
                        pt[:, dj, :],
                        xq[(tm, dg)][:, dj * P:(dj + 1) * P],
                        ident,
                    )
                nc.vector.tensor_copy(
                    xT[:, dg * 4:(dg + 1) * 4, tm * P:(tm + 1) * P], pt
                )

        # ---- remaining weight casts on vector, in arrival order ----
        def cast_w2_e8(g):
            for m in range(4):
                for h2 in range(2):
                    nc.vector.tensor_copy(
                        w2g[g][:, m, h2 * 512:(h2 + 1) * 512],
                        w2e[(g, m, h2)],
                    )

        for g in (1, 4, 2, 5, 3, 6, 7):
            cast_w1_e8(g)
        cast_w2_e8(3)
        cast_w2_e8(0)
        cast_w2_e8(4)
        cast_w2_e8(1)
        cast_w2_e8(5)
        cast_w2_e8(2)
        cast_w2_e8(6)
        cast_w2_e8(7)

        # ---- main pipeline over token chunks ----
        for c in range(NCHUNK):
            # GEMM1 + bias + gelu -> hT[f-part, fo, tok] (bf16)
            hT = hp.tile([P, FO, TC], BF16, tag="hT")
            for fo in range(FO):
                ps = psp.tile([P, TC], F32, tag="ps")
                w1t = w1g[fo // 4]
                fi = (fo % 4) * P
                for do in range(DO):
                    nc.tensor.matmul(
                        ps,
                        w1t[:, do, fi:fi + P],
                        xT[:, do, :],
                        start=(do == 0),
                        stop=(do == DO - 1),
                    )
                nc.scalar.activation(
                    hT[:, fo, :], ps, GELU, bias=b1sb[:, fo:fo + 1]
                )

            # chunk c+1 transposes on PE (bf16): xT is dead now (GEMM1 of
            # chunk c was its last reader, and the PE runs in order).
            if c + 1 < NCHUNK:
                for h in range(2):
                    src = xh[c + 1][h]
                    for dg in range(2):
                        pt = psp.tile(
                            [P, 4, 256], BF16, tag="ps", name=f"ptb{h}_{dg}"
                        )
                        for dj in range(4):
                            do = dg * 4 + dj
                            for tm in range(2):
                                nc.tensor.transpose(
                                    pt[:, dj, tm * P:(tm + 1) * P],
                                    src[:, tm, do * P:(do + 1) * P],
                                    identb,
                                )
                        nc.scalar.activation(
                            xT[:, dg * 4:(dg + 1) * 4, h * 256:(h + 1) * 256],
                            pt,
                            mybir.ActivationFunctionType.Copy,
                        )

            # GEMM2 + bias -> out[tok, d] natural layout
            if c == 0:
                # f-group-major: all 8 psum tiles accumulate per group, so
                # matmuls consume w2 groups as they stream in.
                pts = [
                    psp.tile([P, 512], F32, tag="ps", name=f"ps2_{i}")
                    for i in range(8)
                ]
                for g in range(FG):
                    for tt in range(TC // P):
                        for dh in range(2):
                            pt = pts[tt * 2 + dh]
                            for j in range(4):
                                fo = g * 4 + j
                                nc.tensor.matmul(
                                    pt,
                                    hT[:, fo, tt * P:(tt + 1) * P],
                                    w2g[g][:, j, dh * 512:(dh + 1) * 512],
                                    start=(fo == 0),
                                    stop=(fo == FO - 1),
                                )
                for tt in range(TC // P):
                    for dh in range(2):
                        pt = pts[tt * 2 + dh]
                        osb = op.tile([P, 512], F32, tag="osb")
                        nc.vector.tensor_tensor(
                            osb, pt, b2sb[:, dh * 512:(dh + 1) * 512],
                            mybir.AluOpType.add,
                        )
                        row0 = c * TC + tt * P
                        nc.sync.dma_start(
                            out[row0:row0 + P, dh * 512:(dh + 1) * 512], osb
                        )
            else:
                for tt in range(TC // P):
                    for dh in range(2):
                        ps2t = psp.tile([P, 512], F32, tag="ps")
                        for fo in range(FO):
                            nc.tensor.matmul(
                                ps2t,
                                hT[:, fo, tt * P:(tt + 1) * P],
                                w2g[fo // 4][:, fo % 4, dh * 512:(dh + 1) * 512],
                                start=(fo == 0),
                                stop=(fo == FO - 1),
                            )
                        osb = op.tile([P, 512], F32, tag="osb")
                        nc.vector.tensor_tensor(
                            osb, ps2t, b2sb[:, dh * 512:(dh + 1) * 512],
                            mybir.AluOpType.add,
                        )
                        row0 = c * TC + tt * P
                        nc.sync.dma_start(
                            out[row0:row0 + P, dh * 512:(dh + 1) * 512], osb
                        )


_NC_CACHE = None


def _get_nc():
    global _NC_CACHE
    if _NC_CACHE is None:
        nc = bacc.Bacc(
            "TRN2", target_bir_lowering=False, num_devices=E, num_swdge_queues=4
        )
        x = nc.dram_tensor("x", [TOK, D], F32, kind="ExternalInput").ap()
        w1 = nc.dram_tensor("w1", [D, F], F32, kind="ExternalInput").ap()
        b1 = nc.dram_tensor("b1", [F], F32, kind="ExternalInput").ap()
        w2 = nc.dram_tensor("w2", [F, D], F32, kind="ExternalInput").ap()
        b2 = nc.dram_tensor("b2", [D], F32, kind="ExternalInput").ap()
        out = nc.dram_tensor("out", [TOK, D], F32, kind="ExternalOutput").ap()
        with tile.TileContext(nc) as tctx:
            _build_kernel(tctx, nc, x, w1, b1, w2, b2, out)
        nc.compile()
        _NC_CACHE = nc
    return _NC_CACHE


def kernel(run_opts=None, **inputs):
    x = np.ascontiguousarray(inputs["x"], dtype=np.float32)
    w1 = np.ascontiguousarray(inputs["w1"], dtype=np.float32)
    b1 = np.ascontiguousarray(inputs["b1"], dtype=np.float32)
    w2 = np.ascontiguousarray(inputs["w2"], dtype=np.float32)
    b2 = np.ascontiguousarray(inputs["b2"], dtype=np.float32)

    # x.view(E, B, N, D) in the reference is a pure reshape: expert e owns the
    # contiguous token block e of the flattened (E*B*N, D) buffer.
    xf = x.reshape(E, TOK, D)
    in_maps = [
        {"x": xf[e], "w1": w1[e], "b1": b1[e], "w2": w2[e], "b2": b2[e]}
        for e in range(E)
    ]
    nc = _get_nc()
    res = run_bass_kernel_spmd(
        nc, in_maps, core_ids=list(range(E)), **(run_opts or {})
    )
    outs = np.stack([res.results[e]["out"] for e in range(E)])  # (E, TOK, D)
    if run_opts:
        kernel.last_results = res
    # outputs.view(B, E, N, D) in the reference: reinterpret (E, B*N, D) buffer
    return outs.reshape(B, E, N, D)
